# revision 58
# baseline (speedup 1.0000x reference)
"""Trainium2 Bass kernel for nn_MicroStreamBlock (dual-stream block:
quaternion attention branch + Hamilton-mix MLP branch).

Contract: kernel(**inputs) takes the FULL unsharded inputs (as produced by
reference.setup_inputs) and returns the FULL (4, 2048, 2048) float32 output.
Internally the flattened (8192, 2048) token stream is row-sharded across 8
NeuronCores (1024 rows each; a core pair shares one batch).

Key structural ideas vs the 119us predecessor:
- The v projection GEMM is gone. Attention pools over ALL time, so
  num_g = sum_t e_tg * v_t = (e^T xhat) @ Wv: the device computes the small
  e^T-xhat GEMM (token-contraction) and the host applies Wv in the epilogue.
- Hamilton GEMM1 runs in fp8 e4m3 DoubleRow (both operands host-quantized,
  x*16 / W*64, un-scaled inside the Gelu activation). GEMM2 stays bf16 --
  both in fp8 would blow the 2e-2 error budget (measured 2.3e-2 in sim).
- GEMM1 is emitted FIRST: it only needs ~2MB of DMA (x1dr+f1dr), so the PE
  starts at ~+4us while the qk weight stream loads under it, instead of
  idling ~14us for wdr. Activation tables: warm Sqrt then Gelu, run all 16
  gelus, then a single switch back for the Sqrt/Square z-chains.
- PSUM: tag "pn" carries G1 -> num accumulators -> den (4 banks), tag "pq"
  carries qk rotation -> G2 (4 banks).
- z-chains trail the qk front by 2 chunks; num partial-GEMM MMs for chunk c
  are injected behind chunk c+2's qk MMs; the last two chunks' num MMs hide
  behind the first G2 psum group.
"""

import math
import sys

sys.path.insert(0, "/opt/trn_rl_repo")

import ml_dtypes
import numpy as np

import concourse.bass as bass  # noqa: F401
import concourse.mybir as mybir
import concourse.tile as tile
from concourse import bacc
from concourse.bass_utils import run_bass_kernel_spmd

BF16 = ml_dtypes.bfloat16
F32 = mybir.dt.float32
BF = mybir.dt.bfloat16
AF = mybir.ActivationFunctionType
ALU = mybir.AluOpType
AX = mybir.AxisListType

NCORES = 8
B, T, DIM = 4, 2048, 2048
HALF = DIM // 2          # 1024
HEADS, RANK = 4, 8
NQ = (HALF // HEADS) // 4  # 64
GRP = HEADS * NQ           # 256 quaternion groups per token
ROWS = (B * T) // NCORES   # 1024 rows per core
P = 128
KC = HALF // P             # 8 contraction chunks of 128
TC = ROWS // P             # 8 token chunks of 128
LN_EPS = 1e-5
WSCALE = 64.0              # host fp8 weight pre-scale
XSCALE = 16.0              # host fp8 x1hat pre-scale (GEMM1)
SQH = math.sqrt(0.5)

_CACHE: dict = {}
_LAST_RESULTS = None


def _build_program(with_bias: bool, use_b1: bool):
    nc = bacc.Bacc("TRN2", target_bir_lowering=False, debug=False,
                   num_devices=NCORES)

    FP8 = mybir.dt.float8e4
    # DoubleRow operands [pi, po, free] with contraction index d = po*128+pi.
    x1dr_d = nc.dram_tensor("x1dr", [P, KC, ROWS], FP8, kind="ExternalInput").ap()
    # f1 pre-split into per-jc blocks so each load is fully contiguous
    f1dr_d = nc.dram_tensor("f1dr", [KC, P, KC, P], FP8, kind="ExternalInput").ap()
    xdr_d = nc.dram_tensor("xdr", [P, KC, ROWS], FP8, kind="ExternalInput").ap()
    wdr_d = nc.dram_tensor("wdr", [P, KC, 2 * HALF], FP8, kind="ExternalInput").ap()
    xr_d = nc.dram_tensor("xr", [P, TC, HALF], FP8, kind="ExternalInput").ap()
    f2_d = nc.dram_tensor("f2w", [HALF, HALF], BF, kind="ExternalInput").ap()
    f28_d = nc.dram_tensor("f2dr8", [P, 2, HALF], FP8, kind="ExternalInput").ap()
    if use_b1:
        b1_d = nc.dram_tensor("b1e", [P, KC], F32, kind="ExternalInput").ap()
    if with_bias:
        bqk_d = nc.dram_tensor("bqke", [1, 2 * HALF], BF, kind="ExternalInput").ap()
        b2_d = nc.dram_tensor("b2e", [1, HALF], BF, kind="ExternalInput").ap()
    hout = nc.dram_tensor("hout", [ROWS, HALF], BF, kind="ExternalOutput").ap()
    numout = nc.dram_tensor("numout", [2 * P, HALF], BF,
                            kind="ExternalOutput").ap()
    denout = nc.dram_tensor("denout", [1, 2 * GRP], F32,
                            kind="ExternalOutput").ap()

    with tile.TileContext(nc) as tc:
        with tc.tile_pool(name="sb", bufs=1) as sb, \
             tc.tile_pool(name="ps", bufs=1, space="PSUM") as ps:

            # ---------------- first loads on the scalar DMA queue -------
            # the scalar queue clears its preamble ~2us before sync's, so
            # the PE's first operands land earlier
            x1dr = sb.tile([P, KC, ROWS], FP8, tag="x1dr")
            f1_t8 = [sb.tile([P, KC, P], FP8, tag="f1b", bufs=8,
                             name=f"f1b{jc}") for jc in range(KC)]
            nc.scalar.dma_start(out=f1_t8[0], in_=f1dr_d[0])
            nc.scalar.dma_start(out=x1dr[:, 0:2, :], in_=x1dr_d[:, 0:2, :])

            # ---------------- constants / table warming ----------------
            ones_bf = sb.tile([P, P], BF, tag="ones_bf")
            nc.vector.memset(ones_bf, 1.0)
            sqb = sb.tile([P, 1], F32, tag="sqb")
            nc.vector.memset(sqb, SQH)
            warm = sb.tile([P, 1], F32, tag="warm")
            nc.scalar.activation(warm, sqb, AF.Sqrt)
            nc.scalar.activation(warm, sqb, AF.Gelu)  # leave Gelu table hot

            # PE warm-up: ~5us of throwaway matmuls on resident data flip
            # the HAM clock gate to 8/8 while the first DMAs are in flight,
            # so the real stream starts at 2.4GHz
            dummy = sb.tile([P, 512], BF, tag="dummy")
            nc.vector.memset(dummy, 0.0)
            pdum = ps.tile([P, 512], F32, tag="pq", bufs=4, name="pdum")
            for i in range(12):
                nc.tensor.matmul(pdum, lhsT=ones_bf, rhs=dummy,
                                 start=(i == 0), stop=(i == 11))

            # ---------------- loads: G1 operands first ------------------
            # fine-grained so the first MM fires off ~384KB: f1 jc0 block +
            # x1 kb-pair 0, then x pairs / f1 blocks paced with the loop
            nc.sync.dma_start(out=x1dr[:, 2:4, :], in_=x1dr_d[:, 2:4, :])
            nc.sync.dma_start(out=f1_t8[1], in_=f1dr_d[1])
            nc.sync.dma_start(out=x1dr[:, 4:6, :], in_=x1dr_d[:, 4:6, :])
            nc.sync.dma_start(out=x1dr[:, 6:8, :], in_=x1dr_d[:, 6:8, :])
            for jc in range(2, KC):
                nc.sync.dma_start(out=f1_t8[jc], in_=f1dr_d[jc])
            if use_b1:
                b1cols = sb.tile([P, KC], F32, tag="b1cols")
                nc.sync.dma_start(out=b1cols, in_=b1_d)
            # qk stream loads under G1 compute
            wdr = sb.tile([P, KC, 2 * HALF], FP8, tag="wdr")
            xdr = sb.tile([P, KC, ROWS], FP8, tag="xdr")
            for kb in range(KC // 2):
                s2 = slice(2 * kb, 2 * kb + 2)
                nc.sync.dma_start(out=wdr[:, s2, :], in_=wdr_d[:, s2, :])
            for kb in range(KC // 2):
                s2 = slice(2 * kb, 2 * kb + 2)
                nc.sync.dma_start(out=xdr[:, s2, :], in_=xdr_d[:, s2, :])
            xr = sb.tile([P, TC, HALF], FP8, tag="xr")
            nc.sync.dma_start(out=xr, in_=xr_d)
            # f2: hidden chunks 0,1 as fp8 DoubleRow (weights x64), 2..7 bf16
            f2dr8 = sb.tile([P, 2, HALF], FP8, tag="f28")
            nc.sync.dma_start(out=f2dr8, in_=f28_d)
            f2_t = []
            for k in range(2, KC):
                t = sb.tile([P, HALF], BF, tag="wg", bufs=6, name=f"f2{k}")
                nc.sync.dma_start(out=t, in_=f2_d[k * P:(k + 1) * P, :])
                f2_t.append(t)
            if with_bias:
                bqkr = sb.tile([1, 2 * HALF], BF, tag="bqkr")
                nc.sync.dma_start(out=bqkr, in_=bqk_d)
                b2r = sb.tile([1, HALF], BF, tag="b2r")
                nc.sync.dma_start(out=b2r, in_=b2_d)

            # ---------------- Hamilton GEMM1 (fp8 DR) -------------------
            # h1 = (16 x1hat) @ (64 W1); gelu un-scales by 1/1024.
            # jc outer / m inner / tt inner-2 amortizes each 256-col DR
            # LDWEIGHTS over two 512-cycle matmuls.
            gts = [None] * (2 * KC)
            gt8 = [sb.tile([P, 2 * 512], FP8, tag="gt8", bufs=2,
                           name=f"gt8_{tt}") for tt in range(2)]
            gelus = []
            for jc in range(KC):
                pms = [ps.tile([P, 512], F32, tag="pn", bufs=4,
                               name=f"pg1_{jc}_{tt}") for tt in range(2)]
                for m in range(KC // 2):
                    for tt in range(2):
                        nc.tensor.matmul(
                            pms[tt],
                            lhsT=f1_t8[jc][:, 2 * m:2 * m + 2, :],
                            rhs=x1dr[:, 2 * m:2 * m + 2,
                                     tt * 512:(tt + 1) * 512],
                            start=(m == 0), stop=(m == KC // 2 - 1),
                            perf_mode=mybir.MatmulPerfMode.DoubleRow)
                for tt in range(2):
                    if jc < 2:
                        gt = gt8[tt][:, jc * 512:(jc + 1) * 512]
                    else:
                        gt = sb.tile([P, 512], BF, tag="gt", bufs=12,
                                     name=f"gt{tt}_{jc}")
                        gts[tt * KC + jc] = gt
                    gelus.append(nc.scalar.activation(
                        gt, pms[tt], AF.Gelu, scale=1.0 / (WSCALE * XSCALE),
                        bias=(b1cols[:, jc:jc + 1] if use_b1 else 0.0)))

            # preload the Sqrt/Copy/Square table in the ACT idle window
            # right after the last gelu, so the first qk psum eviction
            # doesn't eat the 1.3us table load on the critical path
            warm2 = sb.tile([P, 1], F32, tag="warm2")
            tsw = nc.scalar.activation(warm2, sqb, AF.Sqrt)
            tile.add_dep_helper(tsw.ins, gelus[-1].ins, sync=False,
                                reason="single table switch out of Gelu")

            # ---------------- stage 1: qk GEMM + attention partials -----
            # z-chains process chunk PAIRS with a pair-friendly eviction
            # layout (col = sec*1024 + chunk*512 + f): wide DVE ops amortize
            # the ~170ns/op dispatch overhead. Engine split per pair:
            # q-squares on ACT, k-squares on gpsimd, the rest on DVE, so
            # every engine stays under the PE's 7.7us/pair budget.
            state = {}
            pnum = [ps.tile([P, 512], F32, tag="pn", bufs=4,
                            name=f"pnum{gf}") for gf in range(4)]
            eacc = sb.tile([P, 2 * GRP], BF, tag="eacc")
            # e' pair tiles (fp8, DoubleRow over token chunks 2t/2t+1):
            # column = s*GRP + g
            e2p = [sb.tile([P, 2 * GRP], FP8, tag="e2p", bufs=4,
                           name=f"e2p{t}") for t in range(4)]
            qk2p = [None] * 4

            def emit_pyr(t):
                qp = qk2p[t]
                # q-side squares split: lo half on ACT, hi half on DVE
                m2q = sb.tile([P, 2 * HALF], BF, tag="m2q", bufs=2,
                              name=f"m2q{t}")
                nc.scalar.activation(m2q[:, 0:HALF], qp[:, 0:HALF], AF.Square)
                nc.vector.tensor_mul(m2q[:, HALF:], qp[:, 2 * HALF:3 * HALF],
                                     qp[:, 2 * HALF:3 * HALF])
                # k-side squares on gpsimd
                m2k = sb.tile([P, 2 * HALF], BF, tag="m2k", bufs=2,
                              name=f"m2k{t}")
                nc.gpsimd.tensor_mul(m2k[:, 0:HALF], qp[:, HALF:2 * HALF],
                                     qp[:, HALF:2 * HALF])
                nc.gpsimd.tensor_mul(m2k[:, HALF:], qp[:, 3 * HALF:],
                                     qp[:, 3 * HALF:])
                pyq = sb.tile([P, HALF], BF, tag="pyq", bufs=2, name=f"pyq{t}")
                nc.vector.tensor_add(pyq, m2q[:, 0:HALF], m2q[:, HALF:])
                pyk = sb.tile([P, HALF], BF, tag="pyk", bufs=2, name=f"pyk{t}")
                nc.vector.tensor_add(pyk, m2k[:, 0:HALF], m2k[:, HALF:])
                sqq = sb.tile([P, 512], BF, tag="ss", bufs=10, name=f"sqq{t}")
                pyqv = pyq.rearrange("p (c h g) -> p c h g", c=2, h=2)
                nc.vector.tensor_add(sqq.rearrange("p (c g) -> p c g", c=2),
                                     pyqv[:, :, 0, :], pyqv[:, :, 1, :])
                skk = sb.tile([P, 512], BF, tag="ss", bufs=10, name=f"skk{t}")
                pykv = pyk.rearrange("p (c h g) -> p c h g", c=2, h=2)
                nc.vector.tensor_add(skk.rearrange("p (c g) -> p c g", c=2),
                                     pykv[:, :, 0, :], pykv[:, :, 1, :])
                ds = sb.tile([P, 512], F32, tag="ds", bufs=2, name=f"ds{t}")
                nc.vector.scalar_tensor_tensor(out=ds, in0=sqq, scalar=1e-12,
                                               in1=skk, op0=ALU.max,
                                               op1=ALU.mult)
                rs = sb.tile([P, 512], F32, tag="rs", bufs=2, name=f"rs{t}")
                nc.vector.reciprocal_approx_fast(rs, ds)
                # cross products
                prA = sb.tile([P, HALF], BF, tag="prA", bufs=2, name=f"prA{t}")
                nc.vector.tensor_mul(prA, qp[:, 0:HALF], qp[:, HALF:2 * HALF])
                prB = sb.tile([P, HALF], BF, tag="prB", bufs=2, name=f"prB{t}")
                nc.vector.tensor_mul(prB, qp[:, 2 * HALF:3 * HALF],
                                     qp[:, 3 * HALF:])
                pa = sb.tile([P, HALF], BF, tag="pa", bufs=2, name=f"pa{t}")
                nc.vector.tensor_add(pa, prA, prB)
                sqk = sb.tile([P, 512], BF, tag="ss", bufs=10, name=f"sqk{t}")
                pav = pa.rearrange("p (c h g) -> p c h g", c=2, h=2)
                nc.vector.tensor_add(sqk.rearrange("p (c g) -> p c g", c=2),
                                     pav[:, :, 0, :], pav[:, :, 1, :])
                state[t] = (sqk, rs)

            zts = [None] * 4

            def emit_isnzt(t):
                # isn/zt have no slow dependencies where emitted: isn's rs is
                # pairs old, zt sits in the DVE queue ahead of later pyramids
                sqk, rs = state[t]
                isn = sb.tile([P, 512], BF, tag="isn", bufs=2, name=f"isn{t}")
                # 1/sqrt(sqq*skk)/8; WSCALE cancels in the cosine
                nc.scalar.activation(isn, rs, AF.Sqrt, scale=1.0 / 64.0)
                zt = sb.tile([P, 512], BF, tag="zt", bufs=4, name=f"zt{t}")
                nc.vector.tensor_mul(zt, sqk, isn)
                zts[t] = zt

            def emit_esq(t):
                # e' = 0.5(z+1)^2 ; emitted late so the ACT FIFO never holds
                # a zt-wait in front of psum evictions the PE needs
                nc.scalar.activation(e2p[t], zts[t], AF.Square, scale=SQH,
                                     bias=sqb)

            def emit_eacc(t):
                # den accumulation on gpsimd, deferred past all num inputs
                # so the slow CAST never delays m2k/zt work in the gp queue
                if t == 0:
                    nc.gpsimd.tensor_copy(eacc, e2p[t])
                else:
                    nc.gpsimd.tensor_add(eacc, eacc, e2p[t])

            def emit_num(t):
                # num partial: pnum[g,f] += e'[pair t]^T @ xr[pair t] (fp8 DR)
                ev = e2p[t].rearrange("p (s g) -> p s g", s=2)
                for g in range(2):
                    for f in range(2):
                        nc.tensor.matmul(
                            pnum[g * 2 + f],
                            lhsT=ev[:, :, g * P:(g + 1) * P],
                            rhs=xr[:, 2 * t:2 * t + 2, f * 512:(f + 1) * 512],
                            start=(t == 0), stop=(t == 3),
                            perf_mode=mybir.MatmulPerfMode.DoubleRow)

            for c in range(TC):
                t, s = c // 2, c % 2
                cs = slice(c * P, (c + 1) * P)
                pss = [ps.tile([P, 512], F32, tag="pq", bufs=4,
                               name=f"ps{c}_{j}") for j in range(4)]
                for kb in range(KC // 2):
                    for j in range(4):
                        nc.tensor.matmul(
                            pss[j],
                            lhsT=xdr[:, 2 * kb:2 * kb + 2, cs],
                            rhs=wdr[:, 2 * kb:2 * kb + 2,
                                    j * 512:(j + 1) * 512],
                            start=(kb == 0),
                            stop=(kb == KC // 2 - 1 and not with_bias),
                            perf_mode=mybir.MatmulPerfMode.DoubleRow)
                if with_bias:
                    for j in range(4):
                        nc.tensor.matmul(
                            pss[j],
                            lhsT=ones_bf[0:1, :],
                            rhs=bqkr[0:1, j * 512:(j + 1) * 512],
                            start=False, stop=True)
                # evictions gate psum slot recycling: emit them first
                if s == 0:
                    qk2p[t] = sb.tile([P, 4 * HALF], BF, tag="qk", bufs=2,
                                      name=f"qk{t}")
                for j in range(4):
                    nc.scalar.copy(
                        qk2p[t][:, j * HALF + s * 512:j * HALF + s * 512 + 512],
                        pss[j])
                if c == 6:
                    emit_isnzt(0)
                if c == 7:
                    # zt(0)/zt(1) land in the DVE FIFO ahead of pair-3's
                    # pyramid and run as soon as pair-2's pyramid drains
                    emit_isnzt(1)
                if s == 1:
                    emit_pyr(t)

            # ---------------- stage 2 front + stage-1 tail --------------
            def gemm2(tcg):
                ht = sb.tile([P, HALF], BF, tag="ht", bufs=3, name=f"h{tcg}")
                tt, t2 = tcg // 4, tcg % 4
                for jj in range(2):
                    pm = ps.tile([P, 512], F32, tag="pq", bufs=4,
                                 name=f"pg2_{tcg}_{jj}")
                    gv = gt8[tt].rearrange("p (s f) -> p s f", s=2)
                    nc.tensor.matmul(
                        pm, lhsT=gv[:, :, t2 * P:(t2 + 1) * P],
                        rhs=f2dr8[:, :, jj * 512:(jj + 1) * 512],
                        start=True, stop=False,
                        perf_mode=mybir.MatmulPerfMode.DoubleRow)
                    for k in range(2, KC):
                        nc.tensor.matmul(
                            pm,
                            lhsT=gts[tt * KC + k][:, t2 * P:(t2 + 1) * P],
                            rhs=f2_t[k - 2][:, jj * 512:(jj + 1) * 512],
                            start=False,
                            stop=(not with_bias and k == KC - 1))
                    if with_bias:
                        nc.tensor.matmul(pm,
                                         lhsT=ones_bf[0:1, :],
                                         rhs=b2r[0:1, jj * 512:(jj + 1) * 512],
                                         start=False, stop=True)
                    # psum carries 64x (f2 pre-scaled for fp8); evictions
                    # divide it back out
                    if tcg == 7 and jj == 1:
                        # final eviction: split DVE+scalar with two DMA
                        # issues to halve the serial tail chain
                        nc.vector.tensor_scalar_mul(ht[:, 512:768],
                                                    pm[:, 0:256], 1.0 / 64)
                        nc.scalar.activation(ht[:, 768:1024], pm[:, 256:512],
                                             AF.Copy, scale=1.0 / 64)
                        nc.sync.dma_start(out=hout[tcg * P:(tcg + 1) * P,
                                                   512:768],
                                          in_=ht[:, 512:768])
                        nc.scalar.dma_start(out=hout[tcg * P:(tcg + 1) * P,
                                                     768:1024],
                                            in_=ht[:, 768:1024])
                    else:
                        # scalar evicts the boundary groups (DVE still busy
                        # with the z-chain tail), DVE the later ones
                        if tcg < 5:
                            nc.scalar.activation(
                                ht[:, jj * 512:(jj + 1) * 512], pm,
                                AF.Copy, scale=1.0 / 64)
                        else:
                            nc.vector.tensor_scalar_mul(
                                ht[:, jj * 512:(jj + 1) * 512], pm, 1.0 / 64)
                        nc.sync.dma_start(
                            out=hout[tcg * P:(tcg + 1) * P,
                                     jj * 512:(jj + 1) * 512],
                            in_=ht[:, jj * 512:(jj + 1) * 512])

            # G2 groups interleave with the last z-chains / num pairs so the
            # PE never waits on the trailing DVE/gpsimd work
            # each num pair sits one G2 group past its input's actual landing
            # time (the DVE finishes pair-3's pyramid ~qk_end+12), so the PE
            # never waits on the trailing z-chain work
            gemm2(0)
            gemm2(1)
            gemm2(2)
            emit_esq(0)
            emit_esq(1)
            emit_num(0)
            gemm2(3)
            emit_isnzt(2)
            emit_num(1)
            gemm2(4)
            emit_isnzt(3)
            emit_esq(2)
            gemm2(5)
            emit_esq(3)
            emit_num(2)
            emit_num(3)
            for t in range(4):
                emit_eacc(t)

            # close out num and ship mid-stage-2
            nsb = sb.tile([P, 2 * HALF], BF, tag="nsb")
            for gf in range(4):
                nc.scalar.copy(nsb[:, gf * 512:(gf + 1) * 512], pnum[gf])
            nc.sync.dma_start(out=numout[0:P, :], in_=nsb[:, 0:HALF])
            nc.sync.dma_start(out=numout[P:2 * P, :], in_=nsb[:, HALF:])

            gemm2(6)

            denp = ps.tile([1, 2 * GRP], F32, tag="pn", bufs=4, name="denp")
            nc.tensor.matmul(denp, lhsT=ones_bf[:, 0:1], rhs=eacc,
                             start=True, stop=True)
            dsb = sb.tile([1, 2 * GRP], F32, tag="dsb")
            nc.scalar.copy(dsb, denp)
            nc.scalar.dma_start(out=denout, in_=dsb)

            gemm2(7)

    nc.compile()
    return nc


def _get_program(with_bias: bool, use_b1: bool):
    key = ("nc", with_bias, use_b1)
    if key not in _CACHE:
        _CACHE[key] = _build_program(with_bias, use_b1)
    return _CACHE[key]


# component-major permutation: new column c*GRP+g <- old column g*4+c
_QPERM = np.arange(HALF).reshape(GRP, 4).T.reshape(-1)


def _dr_pack(a, fp8):
    """[feat, free] -> DoubleRow [pi, kc, free] with feat = kc*128 + pi."""
    return np.ascontiguousarray(
        a.reshape(KC, P, a.shape[1]).transpose(1, 0, 2)).astype(fp8)


def kernel(**inputs) -> np.ndarray:
    x = np.asarray(inputs["x"], np.float32)
    n1_g = np.asarray(inputs["n1_g"], np.float32)
    n1_b = np.asarray(inputs["n1_b"], np.float32)
    wq = np.asarray(inputs["wq"], np.float32)
    bq = np.asarray(inputs["bq"], np.float32)
    wk = np.asarray(inputs["wk"], np.float32)
    bk = np.asarray(inputs["bk"], np.float32)
    wv = np.asarray(inputs["wv"], np.float32)
    bv = np.asarray(inputs["bv"], np.float32)
    wo = np.asarray(inputs["wo"], np.float32)
    bo = np.asarray(inputs["bo"], np.float32)
    n2_g = np.asarray(inputs["n2_g"], np.float32)
    n2_b = np.asarray(inputs["n2_b"], np.float32)
    f1 = np.asarray(inputs["f1"], np.float32)
    b1 = np.asarray(inputs["b1"], np.float32)
    f2 = np.asarray(inputs["f2"], np.float32)
    b2 = np.asarray(inputs["b2"], np.float32)

    isr = 1.0 / math.sqrt(RANK)
    # fold LN affine: gamma into weight rows, beta into effective bias rows
    F1s = f1.sum(0)
    F2s = f2.sum(0)
    W1 = (n2_g[:, None] * F1s) * isr
    b1e = (n2_b @ F1s) * isr + b1
    # q/k columns component-major, interleaved [q_lo k_lo q_hi k_hi]
    Qp = (n1_g[:, None] * wq.T)[:, _QPERM]
    Kp = (n1_g[:, None] * wk.T)[:, _QPERM]
    Wqk = np.concatenate([Qp[:, :512], Kp[:, :512], Qp[:, 512:], Kp[:, 512:]],
                         axis=1)
    bqp = (n1_b @ wq.T + bq)[_QPERM]
    bkp = (n1_b @ wk.T + bk)[_QPERM]
    bqke = np.concatenate([bqp[:512], bkp[:512], bqp[512:], bkp[512:]])

    with_bias = bool(np.any(bqke) or np.any(b2))

    FP8 = np.dtype(mybir.dt.np(mybir.dt.float8e4))
    wdr = _dr_pack(Wqk * WSCALE, FP8)
    # f1 as per-jc contiguous blocks: f1dr[jc, pi, kc, jl]
    f1dr = np.ascontiguousarray(
        _dr_pack(W1 * WSCALE, FP8).reshape(P, KC, KC, P).transpose(2, 0, 1, 3))
    W2s = F2s * (isr * WSCALE)
    f2_bf = W2s.astype(BF16)
    # f2 hidden rows 0:256 DoubleRow-packed fp8: [pi, ko, j], h = ko*128+pi
    f2dr8 = np.ascontiguousarray(
        W2s[0:2 * P].reshape(2, P, HALF).transpose(1, 0, 2)).astype(FP8)
    use_b1 = bool(np.any(b1e))

    xf = np.ascontiguousarray(x.reshape(B * T, DIM))
    shared = {
        "wdr": wdr,
        "f1dr": f1dr,
        "f2w": f2_bf,
        "f2dr8": f2dr8,
    }
    if use_b1:
        # b1cols[p, k] = b1e[k*128+p]
        shared["b1e"] = np.ascontiguousarray(
            b1e.reshape(KC, P).T.astype(np.float32))
    if with_bias:
        shared["bqke"] = np.ascontiguousarray(
            WSCALE * bqke.reshape(1, -1)).astype(BF16)
        shared["b2e"] = np.ascontiguousarray(
            WSCALE * b2.reshape(1, -1)).astype(BF16)

    def _normalize(rows):
        m = rows.mean(1, keepdims=True)
        v = rows.var(1, keepdims=True)
        return (rows - m) / np.sqrt(v + LN_EPS)

    in_maps = []
    xh2_all = []
    for i in range(NCORES):
        rows = xf[i * ROWS:(i + 1) * ROWS]
        m = dict(shared)
        xh1 = _normalize(rows[:, :HALF])            # [tok, feat]
        m["x1dr"] = _dr_pack(np.ascontiguousarray(xh1.T) * XSCALE, FP8)
        xh2 = _normalize(rows[:, HALF:])            # [tok, feat]
        xh2_all.append(xh2)
        m["xdr"] = _dr_pack(np.ascontiguousarray(xh2.T), FP8)
        # xr[p, c, f] = xh2[c*128+p, f]
        m["xr"] = np.ascontiguousarray(
            xh2.reshape(TC, P, HALF).transpose(1, 0, 2)).astype(FP8)
        in_maps.append(m)

    nc = _get_program(with_bias, use_b1)
    res = run_bass_kernel_spmd(nc, in_maps, core_ids=list(range(NCORES)))
    global _LAST_RESULTS
    _LAST_RESULTS = res

    # host epilogue: combine num/den across the core pair, apply Wv and the
    # (4 x d) out-projection, plus both residual adds.
    # device: numout[g, f] = sum_t e'_tg xh2_tf ; denout[g] = sum_t e'_tg
    # with e' = 0.5(z+1)^2; softmax e = e' + 0.5.
    h = np.concatenate([res.results[i]["hout"] for i in range(NCORES)],
                       axis=0).astype(np.float32)
    y2 = xf[:, HALF:] + h
    y1 = np.ascontiguousarray(xf[:, :HALF]).reshape(B, T, HALF)
    WvR = wv.reshape(GRP, 4, HALF)                      # [g, c, f]
    gWv = WvR * n1_g[None, None, :]                     # LN gamma fold
    cv = WvR @ n1_b + bv.reshape(GRP, 4)                # LN beta + bias fold
    for b in range(B):
        A = (res.results[2 * b]["numout"].astype(np.float64)
             + res.results[2 * b + 1]["numout"].astype(np.float64))
        dboth = (res.results[2 * b]["denout"][0].astype(np.float64)
                 + res.results[2 * b + 1]["denout"][0].astype(np.float64))
        den = dboth[:GRP] + dboth[GRP:]
        xsum = xh2_all[2 * b].sum(0) + xh2_all[2 * b + 1].sum(0)
        num = (np.einsum('gf,gcf->gc', A, gWv)
               + den[:, None] * cv
               + 0.5 * (gWv @ xsum + 2 * ROWS * cv))
        dent = den + 0.5 * (2 * ROWS)
        vw = (num / dent[:, None]).reshape(HALF).astype(np.float32)
        y1[b] += vw @ wo.T + bo
    out = np.concatenate([y1.reshape(B * T, HALF), y2], axis=1)
    return np.ascontiguousarray(out.reshape(B, T, DIM))


# revision 59
# speedup vs baseline: 1.0536x; 1.0536x over previous
"""Trainium2 Bass kernel for nn_MicroStreamBlock (dual-stream block:
quaternion attention branch + Hamilton-mix MLP branch).

Contract: kernel(**inputs) takes the FULL unsharded inputs (as produced by
reference.setup_inputs) and returns the FULL (4, 2048, 2048) float32 output.
Internally the flattened (8192, 2048) token stream is row-sharded across 8
NeuronCores (1024 rows each; a core pair shares one batch).

Key structural ideas vs the 119us predecessor:
- The v projection GEMM is gone. Attention pools over ALL time, so
  num_g = sum_t e_tg * v_t = (e^T xhat) @ Wv: the device computes the small
  e^T-xhat GEMM (token-contraction) and the host applies Wv in the epilogue.
- Hamilton GEMM1 runs in fp8 e4m3 DoubleRow (both operands host-quantized,
  x*16 / W*64, un-scaled inside the Gelu activation). GEMM2 stays bf16 --
  both in fp8 would blow the 2e-2 error budget (measured 2.3e-2 in sim).
- GEMM1 is emitted FIRST: it only needs ~2MB of DMA (x1dr+f1dr), so the PE
  starts at ~+4us while the qk weight stream loads under it, instead of
  idling ~14us for wdr. Activation tables: warm Sqrt then Gelu, run all 16
  gelus, then a single switch back for the Sqrt/Square z-chains.
- PSUM: tag "pn" carries G1 -> num accumulators -> den (4 banks), tag "pq"
  carries qk rotation -> G2 (4 banks).
- z-chains trail the qk front by 2 chunks; num partial-GEMM MMs for chunk c
  are injected behind chunk c+2's qk MMs; the last two chunks' num MMs hide
  behind the first G2 psum group.
"""

import math
import sys

sys.path.insert(0, "/opt/trn_rl_repo")

import ml_dtypes
import numpy as np

import concourse.bass as bass  # noqa: F401
import concourse.mybir as mybir
import concourse.tile as tile
from concourse import bacc
from concourse.bass_utils import run_bass_kernel_spmd

BF16 = ml_dtypes.bfloat16
F32 = mybir.dt.float32
BF = mybir.dt.bfloat16
AF = mybir.ActivationFunctionType
ALU = mybir.AluOpType
AX = mybir.AxisListType

NCORES = 8
B, T, DIM = 4, 2048, 2048
HALF = DIM // 2          # 1024
HEADS, RANK = 4, 8
NQ = (HALF // HEADS) // 4  # 64
GRP = HEADS * NQ           # 256 quaternion groups per token
ROWS = (B * T) // NCORES   # 1024 rows per core
P = 128
KC = HALF // P             # 8 contraction chunks of 128
TC = ROWS // P             # 8 token chunks of 128
LN_EPS = 1e-5
WSCALE = 64.0              # host fp8 weight pre-scale
XSCALE = 16.0              # host fp8 x1hat pre-scale (GEMM1)
SQH = math.sqrt(0.5)

_CACHE: dict = {}
_LAST_RESULTS = None


def _build_program(with_bias: bool, use_b1: bool):
    nc = bacc.Bacc("TRN2", target_bir_lowering=False, debug=False,
                   num_devices=NCORES)

    FP8 = mybir.dt.float8e4
    # DoubleRow operands [pi, po, free] with contraction index d = po*128+pi.
    x1dr_d = nc.dram_tensor("x1dr", [P, KC, ROWS], FP8, kind="ExternalInput").ap()
    # f1 pre-split into per-jc blocks so each load is fully contiguous
    f1dr_d = nc.dram_tensor("f1dr", [KC, P, KC, P], FP8, kind="ExternalInput").ap()
    xdr_d = nc.dram_tensor("xdr", [P, KC, ROWS], FP8, kind="ExternalInput").ap()
    wdr_d = nc.dram_tensor("wdr", [P, KC, 2 * HALF], FP8, kind="ExternalInput").ap()
    xr_d = nc.dram_tensor("xr", [P, TC, HALF], FP8, kind="ExternalInput").ap()
    f2_d = nc.dram_tensor("f2w", [HALF, HALF], BF, kind="ExternalInput").ap()
    f28_d = nc.dram_tensor("f2dr8", [P, 2, HALF], FP8, kind="ExternalInput").ap()
    if use_b1:
        b1_d = nc.dram_tensor("b1e", [P, KC], F32, kind="ExternalInput").ap()
    if with_bias:
        bqk_d = nc.dram_tensor("bqke", [1, 2 * HALF], BF, kind="ExternalInput").ap()
        b2_d = nc.dram_tensor("b2e", [1, HALF], BF, kind="ExternalInput").ap()
    hout = nc.dram_tensor("hout", [ROWS, HALF], BF, kind="ExternalOutput").ap()
    numout = nc.dram_tensor("numout", [2 * P, HALF], BF,
                            kind="ExternalOutput").ap()
    denout = nc.dram_tensor("denout", [1, 2 * GRP], F32,
                            kind="ExternalOutput").ap()

    with tile.TileContext(nc) as tc:
        with tc.tile_pool(name="sb", bufs=1) as sb, \
             tc.tile_pool(name="ps", bufs=1, space="PSUM") as ps:

            # ---------------- first loads on the scalar DMA queue -------
            # the scalar queue clears its preamble ~2us before sync's, so
            # the PE's first operands land earlier
            x1dr = sb.tile([P, KC, ROWS], FP8, tag="x1dr")
            f1_t8 = [sb.tile([P, KC, P], FP8, tag="f1b", bufs=8,
                             name=f"f1b{jc}") for jc in range(KC)]
            nc.scalar.dma_start(out=f1_t8[0], in_=f1dr_d[0])
            nc.scalar.dma_start(out=x1dr[:, 0:2, :], in_=x1dr_d[:, 0:2, :])

            # ---------------- constants / table warming ----------------
            ones_bf = sb.tile([P, P], BF, tag="ones_bf")
            nc.vector.memset(ones_bf, 1.0)
            sqb = sb.tile([P, 1], F32, tag="sqb")
            nc.vector.memset(sqb, SQH)
            warm = sb.tile([P, 1], F32, tag="warm")
            nc.scalar.activation(warm, sqb, AF.Sqrt)
            nc.scalar.activation(warm, sqb, AF.Gelu)  # leave Gelu table hot

            # PE warm-up: ~5us of throwaway matmuls on resident data flip
            # the HAM clock gate to 8/8 while the first DMAs are in flight,
            # so the real stream starts at 2.4GHz
            dummy = sb.tile([P, 512], BF, tag="dummy")
            nc.vector.memset(dummy, 0.0)
            pdum = ps.tile([P, 512], F32, tag="pq", bufs=4, name="pdum")
            for i in range(12):
                nc.tensor.matmul(pdum, lhsT=ones_bf, rhs=dummy,
                                 start=(i == 0), stop=(i == 11))

            # ---------------- loads: G1 operands first ------------------
            # fine-grained so the first MM fires off ~384KB: f1 jc0 block +
            # x1 kb-pair 0, then x pairs / f1 blocks paced with the loop
            nc.sync.dma_start(out=x1dr[:, 2:4, :], in_=x1dr_d[:, 2:4, :])
            nc.sync.dma_start(out=f1_t8[1], in_=f1dr_d[1])
            nc.sync.dma_start(out=x1dr[:, 4:6, :], in_=x1dr_d[:, 4:6, :])
            nc.sync.dma_start(out=x1dr[:, 6:8, :], in_=x1dr_d[:, 6:8, :])
            for jc in range(2, KC):
                nc.sync.dma_start(out=f1_t8[jc], in_=f1dr_d[jc])
            if use_b1:
                b1cols = sb.tile([P, KC], F32, tag="b1cols")
                nc.sync.dma_start(out=b1cols, in_=b1_d)
            # qk stream loads under G1 compute
            wdr = sb.tile([P, KC, 2 * HALF], FP8, tag="wdr")
            xdr = sb.tile([P, KC, ROWS], FP8, tag="xdr")
            for kb in range(KC // 2):
                s2 = slice(2 * kb, 2 * kb + 2)
                nc.sync.dma_start(out=wdr[:, s2, :], in_=wdr_d[:, s2, :])
            for kb in range(KC // 2):
                s2 = slice(2 * kb, 2 * kb + 2)
                nc.sync.dma_start(out=xdr[:, s2, :], in_=xdr_d[:, s2, :])
            xr = sb.tile([P, TC, HALF], FP8, tag="xr")
            nc.sync.dma_start(out=xr, in_=xr_d)
            # f2: hidden chunks 0,1 as fp8 DoubleRow (weights x64), 2..7 bf16
            f2dr8 = sb.tile([P, 2, HALF], FP8, tag="f28")
            nc.sync.dma_start(out=f2dr8, in_=f28_d)
            f2_t = []
            for k in range(2, KC):
                t = sb.tile([P, HALF], BF, tag="wg", bufs=6, name=f"f2{k}")
                nc.sync.dma_start(out=t, in_=f2_d[k * P:(k + 1) * P, :])
                f2_t.append(t)
            if with_bias:
                bqkr = sb.tile([1, 2 * HALF], BF, tag="bqkr")
                nc.sync.dma_start(out=bqkr, in_=bqk_d)
                b2r = sb.tile([1, HALF], BF, tag="b2r")
                nc.sync.dma_start(out=b2r, in_=b2_d)

            # ---------------- Hamilton GEMM1 (fp8 DR) -------------------
            # h1 = (16 x1hat) @ (64 W1); gelu un-scales by 1/1024.
            # jc outer / m inner / tt inner-2 amortizes each 256-col DR
            # LDWEIGHTS over two 512-cycle matmuls.
            gts = [None] * (2 * KC)
            gt8 = [sb.tile([P, 2 * 512], FP8, tag="gt8", bufs=2,
                           name=f"gt8_{tt}") for tt in range(2)]
            gelus = []
            for jc in range(KC):
                pms = [ps.tile([P, 512], F32, tag="pn", bufs=4,
                               name=f"pg1_{jc}_{tt}") for tt in range(2)]
                for m in range(KC // 2):
                    for tt in range(2):
                        nc.tensor.matmul(
                            pms[tt],
                            lhsT=f1_t8[jc][:, 2 * m:2 * m + 2, :],
                            rhs=x1dr[:, 2 * m:2 * m + 2,
                                     tt * 512:(tt + 1) * 512],
                            start=(m == 0), stop=(m == KC // 2 - 1),
                            perf_mode=mybir.MatmulPerfMode.DoubleRow)
                for tt in range(2):
                    if jc < 2:
                        gt = gt8[tt][:, jc * 512:(jc + 1) * 512]
                    else:
                        gt = sb.tile([P, 512], BF, tag="gt", bufs=12,
                                     name=f"gt{tt}_{jc}")
                        gts[tt * KC + jc] = gt
                    gelus.append(nc.scalar.activation(
                        gt, pms[tt], AF.Gelu, scale=1.0 / (WSCALE * XSCALE),
                        bias=(b1cols[:, jc:jc + 1] if use_b1 else 0.0)))

            # preload the Sqrt/Copy/Square table in the ACT idle window
            # right after the last gelu, so the first qk psum eviction
            # doesn't eat the 1.3us table load on the critical path
            warm2 = sb.tile([P, 1], F32, tag="warm2")
            tsw = nc.scalar.activation(warm2, sqb, AF.Sqrt)
            tile.add_dep_helper(tsw.ins, gelus[-1].ins, sync=False,
                                reason="single table switch out of Gelu")

            # ---------------- stage 1: qk GEMM + attention partials -----
            # z-chains process chunk PAIRS with a pair-friendly eviction
            # layout (col = sec*1024 + chunk*512 + f): wide DVE ops amortize
            # the ~170ns/op dispatch overhead. Engine split per pair:
            # q-squares on ACT, k-squares on gpsimd, the rest on DVE, so
            # every engine stays under the PE's 7.7us/pair budget.
            state = {}
            pnum = [ps.tile([P, 512], F32, tag="pn", bufs=4,
                            name=f"pnum{gf}") for gf in range(4)]
            eacc = sb.tile([P, 2 * GRP], BF, tag="eacc")
            # e' pair tiles (fp8, DoubleRow over token chunks 2t/2t+1):
            # column = s*GRP + g
            e2p = [sb.tile([P, 2 * GRP], FP8, tag="e2p", bufs=4,
                           name=f"e2p{t}") for t in range(4)]
            qk2p = [None] * 4

            def emit_pyr(t):
                qp = qk2p[t]
                # q-side squares split: lo half on ACT, hi half on DVE
                m2q = sb.tile([P, 2 * HALF], BF, tag="m2q", bufs=2,
                              name=f"m2q{t}")
                nc.scalar.activation(m2q[:, 0:HALF], qp[:, 0:HALF], AF.Square)
                nc.vector.tensor_mul(m2q[:, HALF:], qp[:, 2 * HALF:3 * HALF],
                                     qp[:, 2 * HALF:3 * HALF])
                # k-side squares on gpsimd
                m2k = sb.tile([P, 2 * HALF], BF, tag="m2k", bufs=2,
                              name=f"m2k{t}")
                nc.gpsimd.tensor_mul(m2k[:, 0:HALF], qp[:, HALF:2 * HALF],
                                     qp[:, HALF:2 * HALF])
                nc.gpsimd.tensor_mul(m2k[:, HALF:], qp[:, 3 * HALF:],
                                     qp[:, 3 * HALF:])
                pyq = sb.tile([P, HALF], BF, tag="pyq", bufs=2, name=f"pyq{t}")
                nc.vector.tensor_add(pyq, m2q[:, 0:HALF], m2q[:, HALF:])
                pyk = sb.tile([P, HALF], BF, tag="pyk", bufs=2, name=f"pyk{t}")
                nc.vector.tensor_add(pyk, m2k[:, 0:HALF], m2k[:, HALF:])
                sqq = sb.tile([P, 512], BF, tag="ss", bufs=10, name=f"sqq{t}")
                pyqv = pyq.rearrange("p (c h g) -> p c h g", c=2, h=2)
                nc.vector.tensor_add(sqq.rearrange("p (c g) -> p c g", c=2),
                                     pyqv[:, :, 0, :], pyqv[:, :, 1, :])
                skk = sb.tile([P, 512], BF, tag="ss", bufs=10, name=f"skk{t}")
                pykv = pyk.rearrange("p (c h g) -> p c h g", c=2, h=2)
                nc.vector.tensor_add(skk.rearrange("p (c g) -> p c g", c=2),
                                     pykv[:, :, 0, :], pykv[:, :, 1, :])
                ds = sb.tile([P, 512], F32, tag="ds", bufs=2, name=f"ds{t}")
                nc.vector.scalar_tensor_tensor(out=ds, in0=sqq, scalar=1e-12,
                                               in1=skk, op0=ALU.max,
                                               op1=ALU.mult)
                rs = sb.tile([P, 512], F32, tag="rs", bufs=2, name=f"rs{t}")
                nc.vector.reciprocal_approx_fast(rs, ds)
                # cross products
                prA = sb.tile([P, HALF], BF, tag="prA", bufs=2, name=f"prA{t}")
                nc.vector.tensor_mul(prA, qp[:, 0:HALF], qp[:, HALF:2 * HALF])
                prB = sb.tile([P, HALF], BF, tag="prB", bufs=2, name=f"prB{t}")
                nc.vector.tensor_mul(prB, qp[:, 2 * HALF:3 * HALF],
                                     qp[:, 3 * HALF:])
                pa = sb.tile([P, HALF], BF, tag="pa", bufs=2, name=f"pa{t}")
                nc.vector.tensor_add(pa, prA, prB)
                sqk = sb.tile([P, 512], BF, tag="ss", bufs=10, name=f"sqk{t}")
                pav = pa.rearrange("p (c h g) -> p c h g", c=2, h=2)
                nc.vector.tensor_add(sqk.rearrange("p (c g) -> p c g", c=2),
                                     pav[:, :, 0, :], pav[:, :, 1, :])
                state[t] = (sqk, rs)

            zts = [None] * 4

            def emit_isnzt(t):
                # isn/zt have no slow dependencies where emitted: isn's rs is
                # pairs old, zt sits in the DVE queue ahead of later pyramids
                sqk, rs = state[t]
                isn = sb.tile([P, 512], BF, tag="isn", bufs=2, name=f"isn{t}")
                # 1/sqrt(sqq*skk)/8; WSCALE cancels in the cosine
                nc.scalar.activation(isn, rs, AF.Sqrt, scale=1.0 / 64.0)
                zt = sb.tile([P, 512], BF, tag="zt", bufs=4, name=f"zt{t}")
                nc.vector.tensor_mul(zt, sqk, isn)
                zts[t] = zt

            def emit_esq(t):
                # e' = 0.5(z+1)^2 ; emitted late so the ACT FIFO never holds
                # a zt-wait in front of psum evictions the PE needs
                nc.scalar.activation(e2p[t], zts[t], AF.Square, scale=SQH,
                                     bias=sqb)

            def emit_eacc(t):
                # den accumulation on gpsimd, deferred past all num inputs
                # so the slow CAST never delays m2k/zt work in the gp queue
                if t == 0:
                    nc.gpsimd.tensor_copy(eacc, e2p[t])
                else:
                    nc.gpsimd.tensor_add(eacc, eacc, e2p[t])

            def emit_num(t):
                # num partial: pnum[g,f] += e'[pair t]^T @ xr[pair t] (fp8 DR)
                ev = e2p[t].rearrange("p (s g) -> p s g", s=2)
                for g in range(2):
                    for f in range(2):
                        nc.tensor.matmul(
                            pnum[g * 2 + f],
                            lhsT=ev[:, :, g * P:(g + 1) * P],
                            rhs=xr[:, 2 * t:2 * t + 2, f * 512:(f + 1) * 512],
                            start=(t == 0), stop=(t == 3),
                            perf_mode=mybir.MatmulPerfMode.DoubleRow)

            for c in range(TC):
                t, s = c // 2, c % 2
                cs = slice(c * P, (c + 1) * P)
                pss = [ps.tile([P, 512], F32, tag="pq", bufs=4,
                               name=f"ps{c}_{j}") for j in range(4)]
                for kb in range(KC // 2):
                    for j in range(4):
                        nc.tensor.matmul(
                            pss[j],
                            lhsT=xdr[:, 2 * kb:2 * kb + 2, cs],
                            rhs=wdr[:, 2 * kb:2 * kb + 2,
                                    j * 512:(j + 1) * 512],
                            start=(kb == 0),
                            stop=(kb == KC // 2 - 1 and not with_bias),
                            perf_mode=mybir.MatmulPerfMode.DoubleRow)
                if with_bias:
                    for j in range(4):
                        nc.tensor.matmul(
                            pss[j],
                            lhsT=ones_bf[0:1, :],
                            rhs=bqkr[0:1, j * 512:(j + 1) * 512],
                            start=False, stop=True)
                # evictions gate psum slot recycling: emit them first
                if s == 0:
                    qk2p[t] = sb.tile([P, 4 * HALF], BF, tag="qk", bufs=2,
                                      name=f"qk{t}")
                for j in range(4):
                    nc.scalar.copy(
                        qk2p[t][:, j * HALF + s * 512:j * HALF + s * 512 + 512],
                        pss[j])
                if c == 6:
                    emit_isnzt(0)
                if c == 7:
                    # zt(0)/zt(1) land in the DVE FIFO ahead of pair-3's
                    # pyramid and run as soon as pair-2's pyramid drains
                    emit_isnzt(1)
                if s == 1:
                    emit_pyr(t)

            # ---------------- stage 2 front + stage-1 tail --------------
            def gemm2(tcg):
                ht = sb.tile([P, HALF], BF, tag="ht", bufs=3, name=f"h{tcg}")
                tt, t2 = tcg // 4, tcg % 4
                for jj in range(2):
                    pm = ps.tile([P, 512], F32, tag="pq", bufs=4,
                                 name=f"pg2_{tcg}_{jj}")
                    gv = gt8[tt].rearrange("p (s f) -> p s f", s=2)
                    nc.tensor.matmul(
                        pm, lhsT=gv[:, :, t2 * P:(t2 + 1) * P],
                        rhs=f2dr8[:, :, jj * 512:(jj + 1) * 512],
                        start=True, stop=False,
                        perf_mode=mybir.MatmulPerfMode.DoubleRow)
                    for k in range(2, KC):
                        nc.tensor.matmul(
                            pm,
                            lhsT=gts[tt * KC + k][:, t2 * P:(t2 + 1) * P],
                            rhs=f2_t[k - 2][:, jj * 512:(jj + 1) * 512],
                            start=False,
                            stop=(not with_bias and k == KC - 1))
                    if with_bias:
                        nc.tensor.matmul(pm,
                                         lhsT=ones_bf[0:1, :],
                                         rhs=b2r[0:1, jj * 512:(jj + 1) * 512],
                                         start=False, stop=True)
                    # psum carries 64x (f2 pre-scaled for fp8); evictions
                    # divide it back out
                    if tcg == 7 and jj == 1:
                        # final eviction: split DVE+scalar with two DMA
                        # issues to halve the serial tail chain
                        nc.vector.tensor_scalar_mul(ht[:, 512:768],
                                                    pm[:, 0:256], 1.0 / 64)
                        nc.scalar.activation(ht[:, 768:1024], pm[:, 256:512],
                                             AF.Copy, scale=1.0 / 64)
                        nc.sync.dma_start(out=hout[tcg * P:(tcg + 1) * P,
                                                   512:768],
                                          in_=ht[:, 512:768])
                        nc.scalar.dma_start(out=hout[tcg * P:(tcg + 1) * P,
                                                     768:1024],
                                            in_=ht[:, 768:1024])
                    else:
                        # scalar evicts the boundary groups (DVE still busy
                        # with the z-chain tail), DVE the later ones
                        if tcg < 3:
                            nc.scalar.activation(
                                ht[:, jj * 512:(jj + 1) * 512], pm,
                                AF.Copy, scale=1.0 / 64)
                        else:
                            nc.vector.tensor_scalar_mul(
                                ht[:, jj * 512:(jj + 1) * 512], pm, 1.0 / 64)
                        nc.sync.dma_start(
                            out=hout[tcg * P:(tcg + 1) * P,
                                     jj * 512:(jj + 1) * 512],
                            in_=ht[:, jj * 512:(jj + 1) * 512])

            # G2 groups interleave with the last z-chains / num pairs so the
            # PE never waits on the trailing DVE/gpsimd work
            # each num pair sits one G2 group past its input's actual landing
            # time (the DVE finishes pair-3's pyramid ~qk_end+12), so the PE
            # never waits on the trailing z-chain work
            gemm2(0)
            gemm2(1)
            emit_esq(0)
            emit_esq(1)
            emit_num(0)
            gemm2(2)
            emit_isnzt(2)
            emit_num(1)
            gemm2(3)
            emit_isnzt(3)
            emit_esq(2)
            gemm2(4)
            emit_esq(3)
            emit_num(2)
            gemm2(5)
            emit_num(3)
            for t in range(4):
                emit_eacc(t)

            # close out num and ship mid-stage-2
            nsb = sb.tile([P, 2 * HALF], BF, tag="nsb")
            for gf in range(4):
                nc.scalar.copy(nsb[:, gf * 512:(gf + 1) * 512], pnum[gf])
            nc.sync.dma_start(out=numout[0:P, :], in_=nsb[:, 0:HALF])
            nc.sync.dma_start(out=numout[P:2 * P, :], in_=nsb[:, HALF:])

            gemm2(6)

            denp = ps.tile([1, 2 * GRP], F32, tag="pn", bufs=4, name="denp")
            nc.tensor.matmul(denp, lhsT=ones_bf[:, 0:1], rhs=eacc,
                             start=True, stop=True)
            dsb = sb.tile([1, 2 * GRP], F32, tag="dsb")
            nc.scalar.copy(dsb, denp)
            nc.scalar.dma_start(out=denout, in_=dsb)

            gemm2(7)

    nc.compile()
    return nc


def _get_program(with_bias: bool, use_b1: bool):
    key = ("nc", with_bias, use_b1)
    if key not in _CACHE:
        _CACHE[key] = _build_program(with_bias, use_b1)
    return _CACHE[key]


# component-major permutation: new column c*GRP+g <- old column g*4+c
_QPERM = np.arange(HALF).reshape(GRP, 4).T.reshape(-1)


def _dr_pack(a, fp8):
    """[feat, free] -> DoubleRow [pi, kc, free] with feat = kc*128 + pi."""
    return np.ascontiguousarray(
        a.reshape(KC, P, a.shape[1]).transpose(1, 0, 2)).astype(fp8)


def kernel(**inputs) -> np.ndarray:
    x = np.asarray(inputs["x"], np.float32)
    n1_g = np.asarray(inputs["n1_g"], np.float32)
    n1_b = np.asarray(inputs["n1_b"], np.float32)
    wq = np.asarray(inputs["wq"], np.float32)
    bq = np.asarray(inputs["bq"], np.float32)
    wk = np.asarray(inputs["wk"], np.float32)
    bk = np.asarray(inputs["bk"], np.float32)
    wv = np.asarray(inputs["wv"], np.float32)
    bv = np.asarray(inputs["bv"], np.float32)
    wo = np.asarray(inputs["wo"], np.float32)
    bo = np.asarray(inputs["bo"], np.float32)
    n2_g = np.asarray(inputs["n2_g"], np.float32)
    n2_b = np.asarray(inputs["n2_b"], np.float32)
    f1 = np.asarray(inputs["f1"], np.float32)
    b1 = np.asarray(inputs["b1"], np.float32)
    f2 = np.asarray(inputs["f2"], np.float32)
    b2 = np.asarray(inputs["b2"], np.float32)

    isr = 1.0 / math.sqrt(RANK)
    # fold LN affine: gamma into weight rows, beta into effective bias rows
    F1s = f1.sum(0)
    F2s = f2.sum(0)
    W1 = (n2_g[:, None] * F1s) * isr
    b1e = (n2_b @ F1s) * isr + b1
    # q/k columns component-major, interleaved [q_lo k_lo q_hi k_hi]
    Qp = (n1_g[:, None] * wq.T)[:, _QPERM]
    Kp = (n1_g[:, None] * wk.T)[:, _QPERM]
    Wqk = np.concatenate([Qp[:, :512], Kp[:, :512], Qp[:, 512:], Kp[:, 512:]],
                         axis=1)
    bqp = (n1_b @ wq.T + bq)[_QPERM]
    bkp = (n1_b @ wk.T + bk)[_QPERM]
    bqke = np.concatenate([bqp[:512], bkp[:512], bqp[512:], bkp[512:]])

    with_bias = bool(np.any(bqke) or np.any(b2))

    FP8 = np.dtype(mybir.dt.np(mybir.dt.float8e4))
    wdr = _dr_pack(Wqk * WSCALE, FP8)
    # f1 as per-jc contiguous blocks: f1dr[jc, pi, kc, jl]
    f1dr = np.ascontiguousarray(
        _dr_pack(W1 * WSCALE, FP8).reshape(P, KC, KC, P).transpose(2, 0, 1, 3))
    W2s = F2s * (isr * WSCALE)
    f2_bf = W2s.astype(BF16)
    # f2 hidden rows 0:256 DoubleRow-packed fp8: [pi, ko, j], h = ko*128+pi
    f2dr8 = np.ascontiguousarray(
        W2s[0:2 * P].reshape(2, P, HALF).transpose(1, 0, 2)).astype(FP8)
    use_b1 = bool(np.any(b1e))

    xf = np.ascontiguousarray(x.reshape(B * T, DIM))
    shared = {
        "wdr": wdr,
        "f1dr": f1dr,
        "f2w": f2_bf,
        "f2dr8": f2dr8,
    }
    if use_b1:
        # b1cols[p, k] = b1e[k*128+p]
        shared["b1e"] = np.ascontiguousarray(
            b1e.reshape(KC, P).T.astype(np.float32))
    if with_bias:
        shared["bqke"] = np.ascontiguousarray(
            WSCALE * bqke.reshape(1, -1)).astype(BF16)
        shared["b2e"] = np.ascontiguousarray(
            WSCALE * b2.reshape(1, -1)).astype(BF16)

    def _normalize(rows):
        m = rows.mean(1, keepdims=True)
        v = rows.var(1, keepdims=True)
        return (rows - m) / np.sqrt(v + LN_EPS)

    in_maps = []
    xh2_all = []
    for i in range(NCORES):
        rows = xf[i * ROWS:(i + 1) * ROWS]
        m = dict(shared)
        xh1 = _normalize(rows[:, :HALF])            # [tok, feat]
        m["x1dr"] = _dr_pack(np.ascontiguousarray(xh1.T) * XSCALE, FP8)
        xh2 = _normalize(rows[:, HALF:])            # [tok, feat]
        xh2_all.append(xh2)
        m["xdr"] = _dr_pack(np.ascontiguousarray(xh2.T), FP8)
        # xr[p, c, f] = xh2[c*128+p, f]
        m["xr"] = np.ascontiguousarray(
            xh2.reshape(TC, P, HALF).transpose(1, 0, 2)).astype(FP8)
        in_maps.append(m)

    nc = _get_program(with_bias, use_b1)
    res = run_bass_kernel_spmd(nc, in_maps, core_ids=list(range(NCORES)))
    global _LAST_RESULTS
    _LAST_RESULTS = res

    # host epilogue: combine num/den across the core pair, apply Wv and the
    # (4 x d) out-projection, plus both residual adds.
    # device: numout[g, f] = sum_t e'_tg xh2_tf ; denout[g] = sum_t e'_tg
    # with e' = 0.5(z+1)^2; softmax e = e' + 0.5.
    h = np.concatenate([res.results[i]["hout"] for i in range(NCORES)],
                       axis=0).astype(np.float32)
    y2 = xf[:, HALF:] + h
    y1 = np.ascontiguousarray(xf[:, :HALF]).reshape(B, T, HALF)
    WvR = wv.reshape(GRP, 4, HALF)                      # [g, c, f]
    gWv = WvR * n1_g[None, None, :]                     # LN gamma fold
    cv = WvR @ n1_b + bv.reshape(GRP, 4)                # LN beta + bias fold
    for b in range(B):
        A = (res.results[2 * b]["numout"].astype(np.float64)
             + res.results[2 * b + 1]["numout"].astype(np.float64))
        dboth = (res.results[2 * b]["denout"][0].astype(np.float64)
                 + res.results[2 * b + 1]["denout"][0].astype(np.float64))
        den = dboth[:GRP] + dboth[GRP:]
        xsum = xh2_all[2 * b].sum(0) + xh2_all[2 * b + 1].sum(0)
        num = (np.einsum('gf,gcf->gc', A, gWv)
               + den[:, None] * cv
               + 0.5 * (gWv @ xsum + 2 * ROWS * cv))
        dent = den + 0.5 * (2 * ROWS)
        vw = (num / dent[:, None]).reshape(HALF).astype(np.float32)
        y1[b] += vw @ wo.T + bo
    out = np.concatenate([y1.reshape(B * T, HALF), y2], axis=1)
    return np.ascontiguousarray(out.reshape(B, T, DIM))
